# Initial kernel scaffold
#
"""Distributed causal multi-head attention (QKV projection + flash attention)
for Trainium2, sharded head-parallel across 8 NeuronCores.

Problem: x[2,2048,1024] @ W[1024,3072] + b -> qkv; causal softmax attention
(16 heads, head_dim 64); output [2,2048,16,64].

Sharding: core c handles batch c//4 and the 4 heads 4*(c%4)..4*(c%4)+3.
Each core's output slice is disjoint -> no collectives.

Device kernel (per core, bf16 matmuls with fp32 PSUM accumulation):
  - host passes x pre-transposed (xT [1024,2048] bf16) and W column-sliced,
    reordered and bf16-converted
  - projection: qT/kT produced transposed ([head-pair 128, S]) with W as the
    stationary operand; v produced natural ([S,64] tiles) with xT stationary
  - attention per head-pair: scoresT[sk,sq] = kT.T @ qT row-packed 2 heads per
    PE pass (K=64 each, tile_position rows 0-63 / 64-127) into one 2-bank PSUM
    tile; exp on ACT with scale=1/8 (one fused op for full-width blocks);
    causal via partial-width blocks + a [128,128] triangular additive mask on
    diagonal windows only; PV accumulates outT[65, sq] += v'[sk,65].T @
    expT[sk,sq] where v' has a ones column (DVE memset) -> row 64 = softmax denominator.
  - output: unnormalized [4, 65, 2048] f32; host divides by row 64, adds the
    v bias, transposes into the full output.
"""

import numpy as np

NUM_HEAD = 16
HEAD_DIM = 64
HIDDEN = 1024
B, S = 2, 2048
N_CORES = 8
HPC = 4          # heads per core
NCH = 4          # sq chunks of 512
CHW = 512        # chunk width
NT = 16          # sk tiles of 128
KB = 8           # k-dim blocks of 128
NEG = -1.0e9
SCALE = HEAD_DIM ** -0.5

_CACHE = {}


def _build(repeat=1):
    import concourse.bacc as bacc
    import concourse.mybir as mybir
    import concourse.tile as tile

    f32 = mybir.dt.float32
    bf16 = mybir.dt.bfloat16
    AF = mybir.ActivationFunctionType

    nc = bacc.Bacc("TRN2", target_bir_lowering=False, debug=False)

    XT = nc.dram_tensor("XT", [HIDDEN, S], bf16, kind="ExternalInput")
    WQK = nc.dram_tensor("WQK", [HIDDEN, 512], bf16, kind="ExternalInput")
    WV = nc.dram_tensor("WV", [HIDDEN, 256], bf16, kind="ExternalInput")
    BQKT = nc.dram_tensor("BQKT", [128, 4], f32, kind="ExternalInput")
    TRI = nc.dram_tensor("TRI", [128, 128], f32, kind="ExternalInput")
    OUT = nc.dram_tensor("OUT", [HPC, 65, S], f32, kind="ExternalOutput")

    with tile.TileContext(nc) as tc:
        with tc.tile_pool(name="const", bufs=1) as const_pool, \
             tc.tile_pool(name="qkv", bufs=1) as qkv_pool, \
             tc.tile_pool(name="xt", bufs=4) as xt_pool, \
             tc.tile_pool(name="exps", bufs=10) as exp_pool, \
             tc.tile_pool(name="outs", bufs=4) as out_pool, \
             tc.tile_pool(name="ps_sc", bufs=2, space="PSUM") as ps_sc, \
             tc.tile_pool(name="ps_pr", bufs=2, space="PSUM") as ps_pr, \
             tc.tile_pool(name="ps_pv", bufs=2, space="PSUM") as ps_pv:

            for _rep in range(repeat):
                wqk_sb = const_pool.tile([128, KB, 512], bf16, tag="wqk")
                wv_sb = const_pool.tile([128, KB, 256], bf16, tag="wv")
                bqk_sb = const_pool.tile([128, 4], f32, tag="bqk")
                tri_sb = const_pool.tile([128, 128], f32, tag="tri")

                for kb in range(KB):
                    nc.sync.dma_start(wqk_sb[:, kb, :], WQK[kb * 128:(kb + 1) * 128, :])
                    nc.sync.dma_start(wv_sb[:, kb, :], WV[kb * 128:(kb + 1) * 128, :])
                nc.sync.dma_start(bqk_sb[:], BQKT[:])
                nc.sync.dma_start(tri_sb[:], TRI[:])

                # qT2/kT2: [pair, 128 (2 heads x 64 d), S]; v: [sk-tile, head, 65]
                qT2 = qkv_pool.tile([128, 2, S], bf16, tag="qT2")
                kT2 = qkv_pool.tile([128, 2, S], bf16, tag="kT2")
                v_sb = qkv_pool.tile([128, NT, HPC, 65], bf16, tag="v")
                nc.vector.memset(v_sb[:, :, :, 64], 1.0)

                def emit_xt_dma(C):
                    xt = xt_pool.tile([128, KB, CHW], bf16, tag="xt")
                    for kb in range(KB):
                        nc.gpsimd.dma_start(
                            xt[:, kb, :],
                            XT[kb * 128:(kb + 1) * 128, C * CHW:(C + 1) * CHW])
                    return xt

                def emit_qkT_group(C, xt, blk):
                    # col-blocks: 0,1 = q pair0/pair1; 2,3 = k pair0/pair1
                    ps = ps_pr.tile([128, CHW], f32, tag="pr")
                    for kb in range(KB):
                        nc.tensor.matmul(
                            ps[:],
                            wqk_sb[:, kb, blk * 128:(blk + 1) * 128],
                            xt[:, kb, :],
                            start=(kb == 0), stop=(kb == KB - 1))
                    dest = (qT2 if blk < 2 else kT2)[:, blk % 2,
                                                     C * CHW:(C + 1) * CHW]
                    nc.vector.tensor_scalar_add(dest, ps[:],
                                                bqk_sb[:, blk:blk + 1])

                def emit_v_group(C, xt, rt):
                    t = C * 4 + rt
                    psv = ps_pr.tile([128, 256], f32, tag="pr")
                    for kb in range(KB):
                        nc.tensor.matmul(
                            psv[:],
                            xt[:, kb, rt * 128:(rt + 1) * 128],
                            wv_sb[:, kb, :],
                            start=(kb == 0), stop=(kb == KB - 1))
                    nc.vector.tensor_copy(v_sb[:, t, :, 0:64], psv[:])

                def proj_pair(C, xt, p):
                    # groups needed by pair p's attention: q blk p, k blk 2+p,
                    # plus (for p==0) all v tiles of this chunk
                    emit_qkT_group(C, xt, p)
                    emit_qkT_group(C, xt, 2 + p)
                    if p == 0:
                        for rt in range(4):
                            emit_v_group(C, xt, rt)

                def attn_block(C, p, pvA, pvB, i, nblk):
                    m = i - 4 * C
                    off = 0 if m < 0 else 128 * m
                    w = CHW - off
                    sqs = C * CHW + off
                    psM = ps_sc.tile([128, 2 * CHW], f32, tag="sc")
                    nc.tensor.matmul(
                        psM[:, 0:w],
                        kT2[0:64, p, i * 128:(i + 1) * 128],
                        qT2[0:64, p, sqs:sqs + w],
                        start=True, stop=True, tile_position=(0, 0))
                    nc.tensor.matmul(
                        psM[:, CHW:CHW + w],
                        kT2[64:128, p, i * 128:(i + 1) * 128],
                        qT2[64:128, p, sqs:sqs + w],
                        start=True, stop=True, tile_position=(64, 0))
                    expM = exp_pool.tile([128, 2 * CHW], bf16, tag="exp")
                    if m >= 0:
                        nc.vector.tensor_add(psM[:, 0:128], psM[:, 0:128],
                                             tri_sb[:])
                        nc.vector.tensor_add(psM[:, CHW:CHW + 128],
                                             psM[:, CHW:CHW + 128], tri_sb[:])
                        if m == 1:
                            nc.scalar.activation(expM[:, 0:CHW + w],
                                                 psM[:, 0:CHW + w],
                                                 AF.Exp, scale=SCALE)
                        else:
                            nc.scalar.activation(expM[:, 0:w], psM[:, 0:w],
                                                 AF.Exp, scale=SCALE)
                            nc.scalar.activation(expM[:, CHW:CHW + w],
                                                 psM[:, CHW:CHW + w],
                                                 AF.Exp, scale=SCALE)
                    else:
                        nc.scalar.activation(expM[:], psM[:], AF.Exp,
                                             scale=SCALE)
                    hA, hB = 2 * p, 2 * p + 1
                    nc.tensor.matmul(
                        pvA[0:65, off:CHW], v_sb[:, i, hA, :], expM[:, 0:w],
                        start=(i == 0), stop=(i == nblk - 1))
                    nc.tensor.matmul(
                        pvB[0:65, off:CHW], v_sb[:, i, hB, :],
                        expM[:, CHW:CHW + w],
                        start=(i == 0), stop=(i == nblk - 1))

                def emit_out(C, p, pvA, pvB):
                    hA, hB = 2 * p, 2 * p + 1
                    oA = out_pool.tile([128, CHW], f32, tag="o")
                    oB = out_pool.tile([128, CHW], f32, tag="o")
                    nc.vector.tensor_copy(oA[0:65, :], pvA[0:65, :])
                    nc.vector.tensor_copy(oB[0:65, :], pvB[0:65, :])
                    nc.sync.dma_start(OUT[hA, :, C * CHW:(C + 1) * CHW],
                                      oA[0:65, :])
                    nc.sync.dma_start(OUT[hB, :, C * CHW:(C + 1) * CHW],
                                      oB[0:65, :])

                LAST = -1  # interleaved last-chunk variant measured slower; disabled
                for C in range(NCH):
                    xt_c = emit_xt_dma(C)

                    if C == LAST:
                        # last chunk: no projection remains to fill PE, so
                        # borrow the idle proj psum banks as pair-1's PV
                        # accumulators and interleave both pairs' blocks
                        proj_pair(C, xt_c, 0)
                        proj_pair(C, xt_c, 1)
                        nblk = 4 * C + 4
                        pv0A = ps_pv.tile([128, CHW], f32, tag="pv")
                        pv0B = ps_pv.tile([128, CHW], f32, tag="pv")
                        pv1A = ps_pr.tile([128, CHW], f32, tag="pr")
                        pv1B = ps_pr.tile([128, CHW], f32, tag="pr")
                        for i in range(nblk):
                            attn_block(C, 0, pv0A, pv0B, i, nblk)
                            attn_block(C, 1, pv1A, pv1B, i, nblk)
                        emit_out(C, 0, pv0A, pv0B)
                        emit_out(C, 1, pv1A, pv1B)
                        continue

                    # ---- attention for sq chunk C, both head pairs ----
                    # pair-1's projection is emitted after pair-0's attention
                    # so ACT exps overlap the remaining PE projection work
                    for p in range(2):
                        proj_pair(C, xt_c, p)
                        hA, hB = 2 * p, 2 * p + 1
                        pvA = ps_pv.tile([128, CHW], f32, tag="pv")
                        pvB = ps_pv.tile([128, CHW], f32, tag="pv")
                        nblk = 4 * C + 4

                        def emit_qk(i):
                            m = i - 4 * C
                            off = 0 if m < 0 else 128 * m
                            w = CHW - off
                            sqs = C * CHW + off
                            psM = ps_sc.tile([128, 2 * CHW], f32, tag="sc")
                            nc.tensor.matmul(
                                psM[:, 0:w],
                                kT2[0:64, p, i * 128:(i + 1) * 128],
                                qT2[0:64, p, sqs:sqs + w],
                                start=True, stop=True, tile_position=(0, 0))
                            nc.tensor.matmul(
                                psM[:, CHW:CHW + w],
                                kT2[64:128, p, i * 128:(i + 1) * 128],
                                qT2[64:128, p, sqs:sqs + w],
                                start=True, stop=True, tile_position=(64, 0))
                            return psM, m, off, w

                        def emit_tail(i, psM, m, off, w):
                            expM = exp_pool.tile([128, 2 * CHW], bf16, tag="exp")
                            if m >= 0:
                                nc.vector.tensor_add(psM[:, 0:128],
                                                     psM[:, 0:128], tri_sb[:])
                                nc.vector.tensor_add(psM[:, CHW:CHW + 128],
                                                     psM[:, CHW:CHW + 128],
                                                     tri_sb[:])
                                if m == 1:
                                    nc.scalar.activation(
                                        expM[:, 0:CHW + w], psM[:, 0:CHW + w],
                                        AF.Exp, scale=SCALE)
                                else:
                                    nc.scalar.activation(expM[:, 0:w],
                                                         psM[:, 0:w],
                                                         AF.Exp, scale=SCALE)
                                    nc.scalar.activation(expM[:, CHW:CHW + w],
                                                         psM[:, CHW:CHW + w],
                                                         AF.Exp, scale=SCALE)
                            else:
                                nc.scalar.activation(expM[:], psM[:],
                                                     AF.Exp, scale=SCALE)
                            nc.tensor.matmul(
                                pvA[0:65, off:CHW], v_sb[:, i, hA, :],
                                expM[:, 0:w],
                                start=(i == 0), stop=(i == nblk - 1))
                            nc.tensor.matmul(
                                pvB[0:65, off:CHW], v_sb[:, i, hB, :],
                                expM[:, CHW:CHW + w],
                                start=(i == 0), stop=(i == nblk - 1))

                        pending = None
                        for i in range(nblk):
                            cur = emit_qk(i)
                            if pending is not None:
                                emit_tail(i - 1, *pending)
                            pending = cur
                        emit_tail(nblk - 1, *pending)
                        oA = out_pool.tile([128, CHW], f32, tag="o")
                        oB = out_pool.tile([128, CHW], f32, tag="o")
                        nc.vector.tensor_copy(oA[0:65, :], pvA[0:65, :])
                        nc.vector.tensor_copy(oB[0:65, :], pvB[0:65, :])
                        nc.sync.dma_start(OUT[hA, :, C * CHW:(C + 1) * CHW],
                                          oA[0:65, :])
                        nc.sync.dma_start(OUT[hB, :, C * CHW:(C + 1) * CHW],
                                          oB[0:65, :])

    nc.compile()
    return nc


def _get_nc(repeat=1):
    key = ("nc", repeat)
    if key not in _CACHE:
        _CACHE[key] = _build(repeat)
    return _CACHE[key]


def _prep_inputs(x, W, b):
    import ml_dtypes
    bf16 = ml_dtypes.bfloat16

    x = np.asarray(x, dtype=np.float32)
    W = np.asarray(W, dtype=np.float32)
    b = np.asarray(b, dtype=np.float32)

    W4 = W.reshape(HIDDEN, 3, NUM_HEAD, HEAD_DIM)
    b4 = b.reshape(3, NUM_HEAD, HEAD_DIM)

    xT = [np.ascontiguousarray(x[bi].T).astype(bf16) for bi in range(B)]

    tri = np.where(np.arange(128)[None, :] >= np.arange(128)[:, None],
                   np.float32(0.0), np.float32(NEG)).astype(np.float32)

    in_maps = []
    for c in range(N_CORES):
        bi, g = divmod(c, HPC)
        heads = [4 * g + j for j in range(HPC)]
        wqk = np.concatenate(
            [W4[:, 0, h, :] for h in heads] + [W4[:, 1, h, :] for h in heads],
            axis=1)  # [1024, 512]
        wv = np.concatenate([W4[:, 2, h, :] for h in heads], axis=1)  # [1024,256]
        bqkt = np.stack(
            [np.concatenate([b4[0, heads[0]], b4[0, heads[1]]]),
             np.concatenate([b4[0, heads[2]], b4[0, heads[3]]]),
             np.concatenate([b4[1, heads[0]], b4[1, heads[1]]]),
             np.concatenate([b4[1, heads[2]], b4[1, heads[3]]])],
            axis=1)  # [128, 4]
        in_maps.append({
            "XT": xT[bi],
            "WQK": np.ascontiguousarray(wqk).astype(bf16),
            "WV": np.ascontiguousarray(wv).astype(bf16),
            "BQKT": np.ascontiguousarray(bqkt),
            "TRI": tri,
        })
    return in_maps, b4


def kernel(x, W, b):
    from concourse.bass_utils import run_bass_kernel_spmd

    in_maps, b4 = _prep_inputs(x, W, b)
    nc = _get_nc()
    res = run_bass_kernel_spmd(nc, in_maps, core_ids=list(range(N_CORES)))

    out = np.empty((B, S, NUM_HEAD, HEAD_DIM), dtype=np.float32)
    for c in range(N_CORES):
        bi, g = divmod(c, HPC)
        u = res.results[c]["OUT"]               # [4, 65, 2048]
        o = u[:, :64, :] / u[:, 64:65, :]        # [4, 64, 2048]
        out[bi, :, 4 * g:4 * g + 4, :] = o.transpose(2, 0, 1)
    out += b4[2].reshape(1, 1, NUM_HEAD, HEAD_DIM)
    return out



# revision 1
# speedup vs baseline: 1.0024x; 1.0024x over previous
"""Distributed causal multi-head attention (QKV projection + flash attention)
for Trainium2, sharded head-parallel across 8 NeuronCores.

Problem: x[2,2048,1024] @ W[1024,3072] + b -> qkv; causal softmax attention
(16 heads, head_dim 64); output [2,2048,16,64].

Sharding: core c handles batch c//4 and the 4 heads 4*(c%4)..4*(c%4)+3.
Each core's output slice is disjoint -> no collectives.

Device kernel (per core, bf16 matmuls with fp32 PSUM accumulation):
  - host passes x pre-transposed (xT [1024,2048] bf16) and W column-sliced,
    reordered and bf16-converted
  - projection: qT/kT produced transposed ([head-pair 128, S]) with W as the
    stationary operand; v produced natural ([S,64] tiles) with xT stationary
  - attention per head-pair: scoresT[sk,sq] = kT.T @ qT row-packed 2 heads per
    PE pass (K=64 each, tile_position rows 0-63 / 64-127) into one 2-bank PSUM
    tile; exp on ACT with scale=1/8 (one fused op for full-width blocks);
    causal via partial-width blocks + a [128,128] triangular additive mask on
    diagonal windows only; PV accumulates outT[65, sq] += v'[sk,65].T @
    expT[sk,sq] where v' has a ones column (DVE memset) -> row 64 = softmax denominator.
  - output: unnormalized [4, 65, 2048] f32; host divides by row 64, adds the
    v bias, transposes into the full output.
"""

import numpy as np

NUM_HEAD = 16
HEAD_DIM = 64
HIDDEN = 1024
B, S = 2, 2048
N_CORES = 8
HPC = 4          # heads per core
NCH = 4          # sq chunks of 512
CHW = 512        # chunk width
NT = 16          # sk tiles of 128
KB = 8           # k-dim blocks of 128
NEG = -1.0e9
SCALE = HEAD_DIM ** -0.5

_CACHE = {}


def _build(repeat=1):
    import concourse.bacc as bacc
    import concourse.mybir as mybir
    import concourse.tile as tile

    f32 = mybir.dt.float32
    bf16 = mybir.dt.bfloat16
    AF = mybir.ActivationFunctionType

    nc = bacc.Bacc("TRN2", target_bir_lowering=False, debug=False)

    XT = nc.dram_tensor("XT", [HIDDEN, S], bf16, kind="ExternalInput")
    WQK = nc.dram_tensor("WQK", [HIDDEN, 512], bf16, kind="ExternalInput")
    WV = nc.dram_tensor("WV", [HIDDEN, 256], bf16, kind="ExternalInput")
    BQKT = nc.dram_tensor("BQKT", [128, 4], f32, kind="ExternalInput")
    TRI = nc.dram_tensor("TRI", [128, 128], f32, kind="ExternalInput")
    OUT = nc.dram_tensor("OUT", [HPC, 65, S], f32, kind="ExternalOutput")

    with tile.TileContext(nc) as tc:
        with tc.tile_pool(name="const", bufs=1) as const_pool, \
             tc.tile_pool(name="qkv", bufs=1) as qkv_pool, \
             tc.tile_pool(name="xt", bufs=4) as xt_pool, \
             tc.tile_pool(name="exps", bufs=10) as exp_pool, \
             tc.tile_pool(name="outs", bufs=4) as out_pool, \
             tc.tile_pool(name="ps_sc", bufs=2, space="PSUM") as ps_sc, \
             tc.tile_pool(name="ps_pr", bufs=2, space="PSUM") as ps_pr, \
             tc.tile_pool(name="ps_pv", bufs=2, space="PSUM") as ps_pv:

            for _rep in range(repeat):
                wqk_sb = const_pool.tile([128, KB, 512], bf16, tag="wqk")
                wv_sb = const_pool.tile([128, KB, 256], bf16, tag="wv")
                bqk_sb = const_pool.tile([128, 4], f32, tag="bqk")
                tri_sb = const_pool.tile([128, 128], f32, tag="tri")

                for kb in range(KB):
                    nc.sync.dma_start(wqk_sb[:, kb, :], WQK[kb * 128:(kb + 1) * 128, :])
                    nc.sync.dma_start(wv_sb[:, kb, :], WV[kb * 128:(kb + 1) * 128, :])
                nc.sync.dma_start(bqk_sb[:], BQKT[:])
                nc.sync.dma_start(tri_sb[:], TRI[:])

                # qT2/kT2: [pair, 128 (2 heads x 64 d), S]; v: [sk-tile, head, 65]
                qT2 = qkv_pool.tile([128, 2, S], bf16, tag="qT2")
                kT2 = qkv_pool.tile([128, 2, S], bf16, tag="kT2")
                v_sb = qkv_pool.tile([128, NT, HPC, 65], bf16, tag="v")
                nc.vector.memset(v_sb[:, :, :, 64], 1.0)

                def emit_xt_dma(C):
                    xt = xt_pool.tile([128, KB, CHW], bf16, tag="xt")
                    for kb in range(KB):
                        nc.gpsimd.dma_start(
                            xt[:, kb, :],
                            XT[kb * 128:(kb + 1) * 128, C * CHW:(C + 1) * CHW])
                    return xt

                def emit_qkT_group(C, xt, blk):
                    # col-blocks: 0,1 = q pair0/pair1; 2,3 = k pair0/pair1
                    ps = ps_pr.tile([128, CHW], f32, tag="pr")
                    for kb in range(KB):
                        nc.tensor.matmul(
                            ps[:],
                            wqk_sb[:, kb, blk * 128:(blk + 1) * 128],
                            xt[:, kb, :],
                            start=(kb == 0), stop=(kb == KB - 1))
                    dest = (qT2 if blk < 2 else kT2)[:, blk % 2,
                                                     C * CHW:(C + 1) * CHW]
                    nc.vector.tensor_scalar_add(dest, ps[:],
                                                bqk_sb[:, blk:blk + 1])

                def emit_v_group(C, xt, rt):
                    t = C * 4 + rt
                    psv = ps_pr.tile([128, 256], f32, tag="pr")
                    for kb in range(KB):
                        nc.tensor.matmul(
                            psv[:],
                            xt[:, kb, rt * 128:(rt + 1) * 128],
                            wv_sb[:, kb, :],
                            start=(kb == 0), stop=(kb == KB - 1))
                    nc.vector.tensor_copy(v_sb[:, t, :, 0:64], psv[:])

                def proj_pair(C, xt, p):
                    # groups needed by pair p's attention: q blk p, k blk 2+p,
                    # plus (for p==0) all v tiles of this chunk
                    emit_qkT_group(C, xt, p)
                    emit_qkT_group(C, xt, 2 + p)
                    if p == 0:
                        for rt in range(4):
                            emit_v_group(C, xt, rt)

                def attn_block(C, p, pvA, pvB, i, nblk):
                    m = i - 4 * C
                    off = 0 if m < 0 else 128 * m
                    w = CHW - off
                    sqs = C * CHW + off
                    psM = ps_sc.tile([128, 2 * CHW], f32, tag="sc")
                    nc.tensor.matmul(
                        psM[:, 0:w],
                        kT2[0:64, p, i * 128:(i + 1) * 128],
                        qT2[0:64, p, sqs:sqs + w],
                        start=True, stop=True, tile_position=(0, 0))
                    nc.tensor.matmul(
                        psM[:, CHW:CHW + w],
                        kT2[64:128, p, i * 128:(i + 1) * 128],
                        qT2[64:128, p, sqs:sqs + w],
                        start=True, stop=True, tile_position=(64, 0))
                    expM = exp_pool.tile([128, 2 * CHW], bf16, tag="exp")
                    if m >= 0:
                        nc.vector.tensor_add(psM[:, 0:128], psM[:, 0:128],
                                             tri_sb[:])
                        nc.vector.tensor_add(psM[:, CHW:CHW + 128],
                                             psM[:, CHW:CHW + 128], tri_sb[:])
                        if m == 1:
                            nc.scalar.activation(expM[:, 0:CHW + w],
                                                 psM[:, 0:CHW + w],
                                                 AF.Exp, scale=SCALE)
                        else:
                            nc.scalar.activation(expM[:, 0:w], psM[:, 0:w],
                                                 AF.Exp, scale=SCALE)
                            nc.scalar.activation(expM[:, CHW:CHW + w],
                                                 psM[:, CHW:CHW + w],
                                                 AF.Exp, scale=SCALE)
                    else:
                        nc.scalar.activation(expM[:], psM[:], AF.Exp,
                                             scale=SCALE)
                    hA, hB = 2 * p, 2 * p + 1
                    nc.tensor.matmul(
                        pvA[0:65, off:CHW], v_sb[:, i, hA, :], expM[:, 0:w],
                        start=(i == 0), stop=(i == nblk - 1))
                    nc.tensor.matmul(
                        pvB[0:65, off:CHW], v_sb[:, i, hB, :],
                        expM[:, CHW:CHW + w],
                        start=(i == 0), stop=(i == nblk - 1))

                def emit_out(C, p, pvA, pvB):
                    hA, hB = 2 * p, 2 * p + 1
                    oA = out_pool.tile([128, CHW], f32, tag="o")
                    oB = out_pool.tile([128, CHW], f32, tag="o")
                    nc.vector.tensor_copy(oA[0:65, :], pvA[0:65, :])
                    nc.vector.tensor_copy(oB[0:65, :], pvB[0:65, :])
                    nc.sync.dma_start(OUT[hA, :, C * CHW:(C + 1) * CHW],
                                      oA[0:65, :])
                    nc.sync.dma_start(OUT[hB, :, C * CHW:(C + 1) * CHW],
                                      oB[0:65, :])

                LAST = -1  # interleaved last-chunk variant measured slower; disabled
                for C in range(NCH):
                    xt_c = emit_xt_dma(C)

                    if C == LAST:
                        # last chunk: no projection remains to fill PE, so
                        # borrow the idle proj psum banks as pair-1's PV
                        # accumulators and interleave both pairs' blocks
                        proj_pair(C, xt_c, 0)
                        proj_pair(C, xt_c, 1)
                        nblk = 4 * C + 4
                        pv0A = ps_pv.tile([128, CHW], f32, tag="pv")
                        pv0B = ps_pv.tile([128, CHW], f32, tag="pv")
                        pv1A = ps_pr.tile([128, CHW], f32, tag="pr")
                        pv1B = ps_pr.tile([128, CHW], f32, tag="pr")
                        for i in range(nblk):
                            attn_block(C, 0, pv0A, pv0B, i, nblk)
                            attn_block(C, 1, pv1A, pv1B, i, nblk)
                        emit_out(C, 0, pv0A, pv0B)
                        emit_out(C, 1, pv1A, pv1B)
                        continue

                    # ---- attention for sq chunk C, both head pairs ----
                    # pair-1's projection is emitted after pair-0's attention
                    # so ACT exps overlap the remaining PE projection work
                    for p in range(2):
                        proj_pair(C, xt_c, p)
                        hA, hB = 2 * p, 2 * p + 1
                        pvA = ps_pv.tile([128, CHW], f32, tag="pv")
                        pvB = ps_pv.tile([128, CHW], f32, tag="pv")
                        nblk = 4 * C + 4

                        def emit_qk(i):
                            m = i - 4 * C
                            off = 0 if m < 0 else 128 * m
                            w = CHW - off
                            sqs = C * CHW + off
                            psM = ps_sc.tile([128, 2 * CHW], f32, tag="sc")
                            nc.tensor.matmul(
                                psM[:, 0:w],
                                kT2[0:64, p, i * 128:(i + 1) * 128],
                                qT2[0:64, p, sqs:sqs + w],
                                start=True, stop=True, tile_position=(0, 0))
                            nc.tensor.matmul(
                                psM[:, CHW:CHW + w],
                                kT2[64:128, p, i * 128:(i + 1) * 128],
                                qT2[64:128, p, sqs:sqs + w],
                                start=True, stop=True, tile_position=(64, 0))
                            return psM, m, off, w

                        def emit_tail(i, psM, m, off, w):
                            expM = exp_pool.tile([128, 2 * CHW], bf16, tag="exp")
                            if m >= 0:
                                nc.vector.tensor_add(psM[:, 0:128],
                                                     psM[:, 0:128], tri_sb[:])
                                nc.vector.tensor_add(psM[:, CHW:CHW + 128],
                                                     psM[:, CHW:CHW + 128],
                                                     tri_sb[:])
                                if m == 1:
                                    nc.scalar.activation(
                                        expM[:, 0:CHW + w], psM[:, 0:CHW + w],
                                        AF.Exp, scale=SCALE)
                                else:
                                    nc.scalar.activation(expM[:, 0:w],
                                                         psM[:, 0:w],
                                                         AF.Exp, scale=SCALE)
                                    nc.scalar.activation(expM[:, CHW:CHW + w],
                                                         psM[:, CHW:CHW + w],
                                                         AF.Exp, scale=SCALE)
                            else:
                                nc.scalar.activation(expM[:], psM[:],
                                                     AF.Exp, scale=SCALE)
                            nc.tensor.matmul(
                                pvA[0:65, off:CHW], v_sb[:, i, hA, :],
                                expM[:, 0:w],
                                start=(i == 0), stop=(i == nblk - 1))
                            nc.tensor.matmul(
                                pvB[0:65, off:CHW], v_sb[:, i, hB, :],
                                expM[:, CHW:CHW + w],
                                start=(i == 0), stop=(i == nblk - 1))

                        pending = None
                        for i in range(nblk):
                            cur = emit_qk(i)
                            if pending is not None:
                                emit_tail(i - 1, *pending)
                            pending = cur
                        emit_tail(nblk - 1, *pending)
                        oA = out_pool.tile([128, CHW], f32, tag="o")
                        oB = out_pool.tile([128, CHW], f32, tag="o")
                        nc.vector.tensor_copy(oA[0:65, :], pvA[0:65, :])
                        nc.vector.tensor_copy(oB[0:65, :], pvB[0:65, :])
                        nc.sync.dma_start(OUT[hA, :, C * CHW:(C + 1) * CHW],
                                          oA[0:65, :])
                        nc.sync.dma_start(OUT[hB, :, C * CHW:(C + 1) * CHW],
                                          oB[0:65, :])

    nc.compile()
    return nc


def _get_nc(repeat=1):
    key = ("nc", repeat)
    if key not in _CACHE:
        _CACHE[key] = _build(repeat)
    return _CACHE[key]


def _prep_inputs(x, W, b):
    import ml_dtypes
    bf16 = ml_dtypes.bfloat16

    x = np.asarray(x, dtype=np.float32)
    W = np.asarray(W, dtype=np.float32)
    b = np.asarray(b, dtype=np.float32)

    W4 = W.reshape(HIDDEN, 3, NUM_HEAD, HEAD_DIM)
    b4 = b.reshape(3, NUM_HEAD, HEAD_DIM)

    xT = [np.ascontiguousarray(x[bi].T).astype(bf16) for bi in range(B)]

    tri = np.where(np.arange(128)[None, :] >= np.arange(128)[:, None],
                   np.float32(0.0), np.float32(NEG)).astype(np.float32)

    in_maps = []
    for c in range(N_CORES):
        bi, g = divmod(c, HPC)
        heads = [4 * g + j for j in range(HPC)]
        wqk = np.concatenate(
            [W4[:, 0, h, :] for h in heads] + [W4[:, 1, h, :] for h in heads],
            axis=1)  # [1024, 512]
        wv = np.concatenate([W4[:, 2, h, :] for h in heads], axis=1)  # [1024,256]
        bqkt = np.stack(
            [np.concatenate([b4[0, heads[0]], b4[0, heads[1]]]),
             np.concatenate([b4[0, heads[2]], b4[0, heads[3]]]),
             np.concatenate([b4[1, heads[0]], b4[1, heads[1]]]),
             np.concatenate([b4[1, heads[2]], b4[1, heads[3]]])],
            axis=1)  # [128, 4]
        in_maps.append({
            "XT": xT[bi],
            "WQK": np.ascontiguousarray(wqk).astype(bf16),
            "WV": np.ascontiguousarray(wv).astype(bf16),
            "BQKT": np.ascontiguousarray(bqkt),
            "TRI": tri,
        })
    return in_maps, b4


def kernel(x, W, b):
    from concourse.bass_utils import run_bass_kernel_spmd

    in_maps, b4 = _prep_inputs(x, W, b)
    nc = _get_nc()
    res = run_bass_kernel_spmd(nc, in_maps, core_ids=list(range(N_CORES)))

    out = np.empty((B, S, NUM_HEAD, HEAD_DIM), dtype=np.float32)
    for c in range(N_CORES):
        bi, g = divmod(c, HPC)
        u = res.results[c]["OUT"]               # [4, 65, 2048]
        o = u[:, :64, :] / u[:, 64:65, :]        # [4, 64, 2048]
        out[bi, :, 4 * g:4 * g + 4, :] = o.transpose(2, 0, 1)
    out += b4[2].reshape(1, 1, NUM_HEAD, HEAD_DIM)
    return out

